# revision 20
# baseline (speedup 1.0000x reference)
"""GAT (3 convs) + Set2Set + MLP on 8 Trainium2 NeuronCores.

Sharding: nodes in 8 ranges of 6250; edges (incl self-loops) sharded by dst
range so the per-dst segment softmax is core-local. Per conv every core
computes xl = h @ W_ext in bf16 (attention dots fused as extra columns) and
writes node rows to HBM; the edge phase dma_gathers src rows and dst
attention scalars per 128-edge slab, scales rows by exp(leakyrelu(logit)),
and accumulates messages + softmax denominators with one 258-col bf16 matmul
per slab whose stationary operand is a host-precomputed one-hot dst mask.
h is all-gathered (bf16) between convs. Set2Set+MLP run per-core in f32 on a
16-graph slice.
"""
import os
import sys

import numpy as np
import ml_dtypes

sys.path.insert(0, "/opt/trn_rl_repo")

BF16 = ml_dtypes.bfloat16
FP8 = ml_dtypes.float8_e4m3fn

N, E, F_RAW, D, H, B = 50000, 800000, 9, 128, 2, 128
NUM_CONVS = int(os.environ.get("K_CONVS", "3"))
AGGR_STEPS = int(os.environ.get("K_STEPS", "3"))
NEG_SLOPE = 0.2
NCORES = 8
SHARD = N // NCORES            # 6250
HALF = N // 2                  # 25000
NW = (SHARD + 127) // 128      # 49 windows per core
LASTW = SHARD - (NW - 1) * 128 # 106
ROWS = 384                     # xl row stride (bf16) -> 768B, %256 ok
ROWU = 262                     # used cols: xl0 1 xl1 1 asrc(2) adst(2)
GROW = 384                     # gathered cols per edge row (768B)

_cached = {}


# ---------------------------------------------------------------- patches
def _install_patches():
    import concourse.tile as tile_mod
    from concourse.vector_clock import ScopedClock, VectorClock

    if not getattr(tile_mod.TileContext, "_drain_patched", False):
        def patched(self, tick_clock, wait_clock):
            gc = tick_clock.global_clock
            vals = [gc[p] for p in range(27)]
            for p in [p for p in range(27) if vals[p] > 0]:
                sub = [vals[q] if q == p else 0 for q in range(27)]
                nop = self.nc.sync.nop(nofuse=True, hint="drain_wait_split")
                wait_clock.add_sem_waits(
                    nop.ins, ScopedClock({None: VectorClock(sub)}))
            self.nc.sync.drain()
            self.nc.all_engine_barrier()
            popped = self.nc._tile_sem_poison_stack.pop()
            assert popped is self._sem_poison
            self.nc.clear_and_free_semaphores(
                list(self.sems.allocated().values()))
            self.nc.all_engine_barrier()

        tile_mod.TileContext._drain_and_barrier = patched
        tile_mod.TileContext._drain_patched = True


def _split_waits(nc, max_waits=1):
    """walrus here allows at most one sync-wait command per instruction;
    spread extras across injected same-engine NoOps."""
    from concourse import mybir
    n = 0
    for f in nc.m.functions:
        for bb in f.blocks:
            changed, new = False, []
            for ins in bb.instructions:
                si = ins.sync_info
                if si is not None and len(si.on_wait) > max_waits:
                    waits = list(si.on_wait)
                    for i, w in enumerate(waits[max_waits:]):
                        nop = mybir.InstNoOp(
                            name=f"{ins.name}-ws{i}", ins=[], outs=[])
                        nop.engine = ins.engine
                        nop.sync_info = mybir.SyncInfo(
                            on_wait=[w], on_update=[])
                        new.append(nop)
                    ins.sync_info = mybir.SyncInfo(
                        on_wait=waits[:max_waits],
                        on_update=list(si.on_update))
                    changed = True
                    n += 1
                new.append(ins)
            if changed:
                bb.instructions = new
    return n


# ---------------------------------------------------------------- host prep
def _wrap16(flat):
    """dma_gather index layout: idx k at [k%16, k//16], replicated to 128."""
    k = flat.shape[0]
    w = flat.reshape(k // 16, 16).T.astype(np.int16)
    return np.tile(w, (8, 1))


def _host_prep(x, edge_index, batch_index, gat_W, gat_att_src, gat_att_dst):
    cfg = {}
    src = np.concatenate([edge_index[0], np.arange(N, dtype=np.int64)])
    dst = np.concatenate([edge_index[1], np.arange(N, dtype=np.int64)])
    src = src.astype(np.int32)
    dst = dst.astype(np.int32)

    # per (core, window, half): edge lists; half = src >= HALF
    core_of = dst // SHARD
    win_of = (dst % SHARD) // 128
    half_of = (src >= HALF).astype(np.int32)
    key = ((core_of * NW + win_of) * 2 + half_of)
    korder = np.argsort(key, kind="stable")
    src_s, dst_s, key_s = src[korder], dst[korder], key[korder]
    counts = np.bincount(key_s, minlength=NCORES * NW * 2).reshape(
        NCORES, NW, 2)
    # per-window slab counts, padded to the max across cores so one program
    # works for all 8 cores
    SA = np.ceil(counts[:, :, 0].max(axis=0) / 128).astype(int)  # [NW]
    SB = np.ceil(counts[:, :, 1].max(axis=0) / 128).astype(int)
    SW = SA + SB
    offA = np.zeros(NW + 1, int); np.cumsum(SA, out=offA[1:])
    offB = np.zeros(NW + 1, int); np.cumsum(SB, out=offB[1:])
    offW = np.zeros(NW + 1, int); np.cumsum(SW, out=offW[1:])
    cfg["SA"], cfg["SB"], cfg["SW"] = SA, SB, SW
    cfg["offW"] = offW
    TOT = int(offW[-1])
    cfg["TOT"] = TOT
    # merged idx tensor: per window [A | B | awsb] blocks of 8 cols per slab
    offI = np.zeros(NW + 1, int)
    np.cumsum(SW + 1, out=offI[1:])   # SA+SB slabs + 1 awsb block
    cfg["offI"] = offI
    TOTI = int(offI[-1])
    cfg["TOTI"] = TOTI

    starts = np.zeros(NCORES * NW * 2 + 1, np.int64)
    np.cumsum(np.bincount(key_s, minlength=NCORES * NW * 2), out=starts[1:])

    # graph boundaries for set2set
    goff = np.searchsorted(batch_index, np.arange(B + 1))
    rows_per_core = np.array(
        [goff[16 * (c + 1)] - goff[16 * c] for c in range(NCORES)])
    T = int(np.ceil(rows_per_core.max() / 128))
    cfg["T"] = T

    # fused weights: xl row = [xl0, 1, xl1, 1, asrc(2), adst(2)]
    W = np.asarray(gat_W, np.float32)              # [128, 256]
    asrc_v = np.asarray(gat_att_src, np.float32)   # [2, 128]
    adst_v = np.asarray(gat_att_dst, np.float32)
    w_as = np.stack([W[:, h * D:(h + 1) * D] @ asrc_v[h] for h in range(H)],
                    axis=1)                        # [128, 2]
    w_ad = np.stack([W[:, h * D:(h + 1) * D] @ adst_v[h] for h in range(H)],
                    axis=1)
    W_eff = np.zeros((D, ROWU), np.float32)
    W_eff[:, 0:128] = W[:, 0:128]
    W_eff[:, 129:257] = W[:, 128:256]
    W_eff[:, 258:260] = w_as
    W_eff[:, 260:262] = w_ad
    cfg["W_eff"] = W_eff.astype(BF16)

    xp = np.zeros((N, D), np.float32)
    xp[:, :F_RAW] = x
    h0T = np.zeros((NCORES * 128, SHARD), np.float32)
    for s in range(NCORES):
        h0T[128 * s:128 * (s + 1), :] = xp[SHARD * s:SHARD * (s + 1)].T
    cfg["h0T"] = h0T.astype(BF16)

    iota128 = np.arange(128, dtype=np.int32)
    per_core = []
    for c in range(NCORES):
        IDX = np.zeros((128, TOTI * 8), np.int16)
        ind = np.zeros((128, TOT * 256), FP8)
        for w in range(NW):
            dloc = np.full(SW[w] * 128, -1, np.int64)  # dst - 128*w local
            io = offI[w] * 8
            for hf in range(2):
                k = (c * NW + w) * 2 + hf
                lo, hi = starts[k], starts[k + 1]
                cnt = hi - lo
                S_h = (SA[w], SB[w])[hf]
                sl = slice(SA[w] * 128, SA[w] * 128 + S_h * 128) if hf else \
                    slice(0, S_h * 128)
                flat = np.zeros(S_h * 128, np.int64)  # pad -> row 0 (finite)
                flat[:cnt] = src_s[lo:hi] - HALF * hf
                IDX[:, io:io + S_h * 8] = _wrap16(flat)
                io += S_h * 8
                dloc[sl][:cnt] = dst_s[lo:hi] % SHARD - 128 * w
            wrows = (c * SHARD + 128 * w + np.arange(128)) % HALF
            IDX[:, io:io + 8] = _wrap16(np.minimum(wrows, HALF - 1))
            # one-hot masks [128, SW*128 fwd | SW*128 transposed]
            oh = (dloc[:, None] == iota128[None, :])
            o3 = oh.reshape(SW[w], 128, 128)
            ind[:, offW[w] * 256:offW[w] * 256 + SW[w] * 128] = (
                o3.transpose(1, 0, 2).reshape(128, SW[w] * 128).astype(FP8))
            ind[:, offW[w] * 256 + SW[w] * 128:(offW[w] + SW[w]) * 256] = (
                o3.transpose(2, 0, 1).reshape(128, SW[w] * 128).astype(FP8))

        # set2set slice (rows gathered from h3 halves + select)
        r0, r1 = goff[16 * c], goff[16 * (c + 1)]
        rows = np.arange(T * 128)
        glob = np.minimum(r0 + rows, N - 1)
        xidxA = _wrap16(np.minimum(glob, HALF - 1))
        xidxB = _wrap16(np.clip(glob - HALF, 0, HALF - 1))
        sel = (glob < HALF).astype(np.float32)  # 1 -> A half
        selrep = np.tile(
            sel.reshape(T, 128, 1), (1, 1, 128)).transpose(1, 0, 2).astype(
            BF16).reshape(128, T * 128)
        valid = (r0 + rows) < r1
        bl = np.full(T * 128, -1.0, np.float32)
        bl[valid] = (batch_index[glob[valid]] - 16 * c).astype(np.float32)
        bloc = bl.reshape(T, 128, 1).copy()
        brep = np.tile(bl.reshape(T, 1, 128), (1, 16, 1)).astype(np.float32)

        per_core.append(dict(
            IDX=IDX, ind_d=ind,
            selA=np.full((128, 1), 1.0 if c < 4 else 0.0, np.float32),
            s2s_xidxA=xidxA, s2s_xidxB=xidxB, s2s_selrep=selrep,
            s2s_bloc=bloc, s2s_brep=brep,
        ))
    return cfg, per_core


# ---------------------------------------------------------------- device build
def _build(cfg):
    import concourse.bacc as bacc
    import concourse.bass as bass
    import concourse.tile as tile
    from concourse import mybir
    from concourse.masks import make_identity

    _install_patches()
    f32 = mybir.dt.float32
    bf16 = mybir.dt.bfloat16
    i16 = mybir.dt.int16
    f8 = mybir.dt.float8e4
    AF = mybir.ActivationFunctionType
    OP = mybir.AluOpType
    SA, SB, SW = cfg["SA"], cfg["SB"], cfg["SW"]
    offW, offI = cfg["offW"], cfg["offI"]
    TOT, TOTI, T = cfg["TOT"], cfg["TOTI"], cfg["T"]

    nc = bacc.Bacc("TRN2", num_swdge_queues=4)
    P_ = nc.declare_dram_parameter
    h0T = P_("h0T", [NCORES * 128, SHARD], bf16, isOutput=False)
    W_eff = P_("W_eff", [D, ROWU], bf16, isOutput=False)
    bias_rep = P_("bias_rep", [128, 128], f32, isOutput=False)
    IDX = P_("IDX", [128, TOTI * 8], i16, isOutput=False)
    ind_d = P_("ind_d", [128, TOT * 256], f8, isOutput=False)
    selA_p = P_("selA", [128, 1], f32, isOutput=False)
    s2s_xidxA = P_("s2s_xidxA", [128, T * 8], i16, isOutput=False)
    s2s_xidxB = P_("s2s_xidxB", [128, T * 8], i16, isOutput=False)
    s2s_selrep = P_("s2s_selrep", [128, T * 128], bf16, isOutput=False)
    s2s_bloc = P_("s2s_bloc", [T, 128, 1], f32, isOutput=False)
    s2s_brep = P_("s2s_brep", [T, 16, 128], f32, isOutput=False)
    WihT_a = P_("WihT_a", [128, 512], f32, isOutput=False)
    WihT_b = P_("WihT_b", [128, 512], f32, isOutput=False)
    WhhT = P_("WhhT", [128, 512], f32, isOutput=False)
    bg_rep = P_("bg_rep", [16, 512], f32, isOutput=False)
    W1a = P_("W1a", [128, 128], f32, isOutput=False)
    W1b = P_("W1b", [128, 128], f32, isOutput=False)
    W2 = P_("W2", [128, 128], f32, isOutput=False)
    b1_rep = P_("b1_rep", [16, 128], f32, isOutput=False)
    b2_rep = P_("b2_rep", [16, 128], f32, isOutput=False)
    out = P_("out", [16, 128], f32, isOutput=True)

    xlA = nc.dram_tensor("xlA", [HALF, ROWS], bf16)
    xlB = nc.dram_tensor("xlB", [HALF, ROWS], bf16)
    # adst mirror: row r cols 0:2 = adst(node r), cols 2:4 = adst(node r+HALF)
    adst2 = nc.dram_tensor("adst2", [HALF, 128], bf16)
    h_shT = nc.dram_tensor("h_shT", [128, SHARD], bf16)
    ag_hT = nc.dram_tensor("ag_hT", [NCORES * 128, SHARD], bf16,
                           addr_space="Shared")
    h_sh = nc.dram_tensor("h_sh", [SHARD, 128], bf16)
    h3_full = nc.dram_tensor("h3_full", [N, 128], bf16,
                             addr_space="Shared")

    with tile.TileContext(nc) as tc:
        with tc.tile_pool(name="consts", bufs=1) as cp:
            ident = cp.tile([128, 128], f32)
            make_identity(nc, ident[:])
            identb = cp.tile([128, 128], bf16)
            make_identity(nc, identb[:])
            iota16_row = cp.tile([128, 16], f32)
            nc.gpsimd.iota(iota16_row[:], pattern=[[1, 16]], base=0,
                           channel_multiplier=0,
                           allow_small_or_imprecise_dtypes=True)
            iota16_col = cp.tile([16, 1], f32)
            nc.gpsimd.iota(iota16_col[:], pattern=[[0, 1]], base=0,
                           channel_multiplier=1,
                           allow_small_or_imprecise_dtypes=True)
            negones_row = cp.tile([1, 128], f32)
            nc.vector.memset(negones_row[:], -1.0)
            weff_sb = cp.tile([128, ROWU], bf16)
            nc.sync.dma_start(out=weff_sb[:], in_=W_eff[:])
            bias_sb = cp.tile([128, 128], f32)
            nc.sync.dma_start(out=bias_sb[:], in_=bias_rep[:])
            selA_sb = cp.tile([128, 1], f32)
            nc.sync.dma_start(out=selA_sb[:], in_=selA_p[:])
            wia = cp.tile([128, 512], f32)
            nc.sync.dma_start(out=wia[:], in_=WihT_a[:])
            wib = cp.tile([128, 512], f32)
            nc.sync.dma_start(out=wib[:], in_=WihT_b[:])
            whh = cp.tile([128, 512], f32)
            nc.sync.dma_start(out=whh[:], in_=WhhT[:])
            bg_sb = cp.tile([16, 512], f32)
            nc.sync.dma_start(out=bg_sb[:], in_=bg_rep[:])
            w1a_sb = cp.tile([128, 128], f32)
            nc.sync.dma_start(out=w1a_sb[:], in_=W1a[:])
            w1b_sb = cp.tile([128, 128], f32)
            nc.sync.dma_start(out=w1b_sb[:], in_=W1b[:])
            w2_sb = cp.tile([128, 128], f32)
            nc.sync.dma_start(out=w2_sb[:], in_=W2[:])
            b1_sb = cp.tile([16, 128], f32)
            nc.sync.dma_start(out=b1_sb[:], in_=b1_rep[:])
            b2_sb = cp.tile([16, 128], f32)
            nc.sync.dma_start(out=b2_sb[:], in_=b2_rep[:])

            # one-time zero init of gather pad regions (xl cols 256:384,
            # adst2 cols 0:128) so gathered bytes are always finite
            zt = cp.tile([128, 8, 128], bf16)
            nc.vector.memset(zt[:], 0.0)
            eps_sb = cp.tile([128, 1], f32)
            nc.vector.memset(eps_sb[:], 1e-16)
            eps16 = cp.tile([16, 1], f32)
            nc.vector.memset(eps16[:], 1e-16)
            nfull = HALF // 128          # 195
            ntail = HALF - nfull * 128   # 40
            for i in range(0, nfull, 8):
                nb = min(8, nfull - i)
                r0 = 128 * i
                for dstt, c0 in ((xlA, 256), (xlB, 256), (adst2, 0)):
                    nc.sync.dma_start(
                        out=dstt[r0:r0 + 128 * nb, c0:c0 + 128]
                        .rearrange("(s p) c -> p s c", p=128),
                        in_=zt[:, 0:nb, :])
            for dstt, c0 in ((xlA, 256), (xlB, 256), (adst2, 0)):
                nc.sync.dma_start(
                    out=dstt[128 * nfull:HALF, c0:c0 + 128],
                    in_=zt[0:ntail, 0, :])

            qctr = [0]

            def qc():
                qn = qctr[0] % 4
                qctr[0] += 1
                return qn

            for conv in range(NUM_CONVS):
                hT_src = h0T if conv == 0 else ag_hT
                # ---- phase 1: xl = h @ W_eff for all N nodes ----
                # 2 windows per tile; shard s -> adst2 col 2*(s>=4)
                with tc.tile_pool(name="p1s", bufs=4) as p1s, \
                     tc.tile_pool(name="p1p", bufs=4, space="PSUM") as p1p:
                    for s in range(NCORES):
                        xl_half, rbase = (xlA, SHARD * s) if s < 4 else \
                                         (xlB, SHARD * s - HALF)
                        acol = 0 if s < 4 else 2
                        for t2 in range(0, NW, 2):
                            nt = min(2, NW - t2)
                            full = (t2 + nt < NW) or (nt == 2 and
                                                      NW * 128 == SHARD)
                            ncols = 128 * nt if t2 + nt < NW else \
                                128 * (nt - 1) + LASTW
                            hT_t = p1s.tile([128, 256], bf16, tag="hT")
                            nc.scalar.dma_start(
                                out=hT_t[:, 0:ncols],
                                in_=hT_src[128 * s:128 * (s + 1),
                                           128 * t2:128 * t2 + ncols])
                            xo = p1s.tile([128, 2, ROWU], bf16, tag="xo")
                            for k in range(nt):
                                nk = min(128, ncols - 128 * k)
                                ps = p1p.tile([128, ROWU], f32,
                                              tag=f"p1{k}")
                                nc.tensor.matmul(
                                    ps[0:nk, :],
                                    lhsT=hT_t[:, 128 * k:128 * k + nk],
                                    rhs=weff_sb[:], start=True, stop=True)
                                if (t2 + k) % 2 == 0:
                                    nc.scalar.activation(xo[0:nk, k, :],
                                                         ps[0:nk, :],
                                                         AF.Copy)
                                else:
                                    nc.vector.tensor_copy(xo[0:nk, k, :],
                                                          ps[0:nk, :])
                            nc.vector.memset(
                                xo[:, 0:nt, 0:258].rearrange(
                                    "p s (a b) -> p s a b", b=129)[
                                    :, :, :, 128:129], 1.0)
                            # node rows: r = rbase + 128*t2 + k*128 + p
                            nr = 128 * (nt - 1) + (128 if t2 + nt < NW
                                                   else LASTW)
                            nc.sync.dma_start(
                                out=xl_half[rbase + 128 * t2:
                                            rbase + 128 * t2 + nr, 0:ROWU]
                                .rearrange("(s p) c -> p s c", p=128)
                                if nt == 2 and nr == 256 else
                                xl_half[rbase + 128 * t2:
                                        rbase + 128 * t2 + nr, 0:ROWU],
                                in_=xo[:, 0:nt, :] if (nt == 2 and nr == 256)
                                else xo[0:nr, 0, :])
                            nc.sync.dma_start(
                                out=adst2[rbase + 128 * t2:
                                          rbase + 128 * t2 + nr,
                                          acol:acol + 2]
                                .rearrange("(s p) c -> p s c", p=128)
                                if nt == 2 and nr == 256 else
                                adst2[rbase + 128 * t2:
                                      rbase + 128 * t2 + nr, acol:acol + 2],
                                in_=xo[:, 0:nt, 260:262]
                                if (nt == 2 and nr == 256)
                                else xo[0:nr, 0, 260:262])

                # ---- edge phase: one window of 128 dst nodes at a time ----
                with tc.tile_pool(name="eg", bufs=4) as eg, \
                     tc.tile_pool(name="es", bufs=4) as es, \
                     tc.tile_pool(name="em", bufs=4) as em, \
                     tc.tile_pool(name="agg", bufs=3, space="PSUM") as aggp, \
                     tc.tile_pool(name="pep", bufs=2, space="PSUM") as pep, \
                     tc.tile_pool(name="etp", bufs=2, space="PSUM") as etp:
                    for w in range(NW):
                        nwn = 128 if w < NW - 1 else LASTW
                        SA_w, SB_w, SW_w = int(SA[w]), int(SB[w]), int(SW[w])
                        iAll = es.tile([128, (SW_w + 1) * 8], i16,
                                       tag="iAll")
                        nc.sync.dma_start(
                            out=iAll[:],
                            in_=IDX[:, offI[w] * 8:(offI[w] + SW_w + 1) * 8])
                        ind_sb = es.tile([128, SW_w * 256], f8, tag="ind")
                        nc.sync.dma_start(
                            out=ind_sb[:],
                            in_=ind_d[:, offW[w] * 256:(offW[w] + SW_w) * 256])

                        g = eg.tile([128, SW_w, GROW], bf16, tag="g")
                        for (base, S_h, srct) in ((0, SA_w, xlA),
                                                  (SA_w, SB_w, xlB)):
                            s0 = 0
                            while s0 < S_h:
                                ns = min(8, S_h - s0)
                                nc.gpsimd.dma_gather(
                                    out_ap=g[:, base + s0:base + s0 + ns, :],
                                    in_ap=srct[:, 0:GROW],
                                    idxs_ap=iAll[:, (base + s0) * 8:
                                                 (base + s0 + ns) * 8],
                                    num_idxs=ns * 128,
                                    num_idxs_reg=ns * 128, elem_size=GROW,
                                    elem_step=ROWS, queue_num=qc())
                                s0 += ns
                        # window dst-node attention scalars (128 rows of
                        # adst2; cols 0:2 for cores 0-3, 2:4 for 4-7)
                        aw = eg.tile([128, 1, 128], bf16, tag="aw")
                        nc.gpsimd.dma_gather(
                            out_ap=aw[:], in_ap=adst2[:, 0:128],
                            idxs_ap=iAll[:, SW_w * 8:(SW_w + 1) * 8],
                            num_idxs=128, num_idxs_reg=128, elem_size=128,
                            elem_step=128, queue_num=qc())
                        awd = em.tile([128, 2], f32, tag="awd")
                        nc.vector.tensor_tensor(
                            out=awd[:], in0=aw[:, 0, 0:2],
                            in1=aw[:, 0, 2:4], op=OP.subtract)
                        nc.vector.tensor_scalar(
                            out=awd[:], in0=awd[:],
                            scalar1=selA_sb[:, 0:1], scalar2=None,
                            op0=OP.mult)
                        asel = em.tile([128, 2], bf16, tag="asel")
                        nc.vector.tensor_tensor(
                            out=asel[:], in0=awd[:], in1=aw[:, 0, 2:4],
                            op=OP.add)
                        # a_dst per edge: SW tiny matmuls vs transposed masks
                        pe_all = pep.tile([128, 2 * SW_w], f32, tag="pe")
                        for s_ in range(SW_w):
                            nc.tensor.matmul(
                                pe_all[:, 2 * s_:2 * s_ + 2],
                                lhsT=ind_sb[:, (SW_w + s_) * 128:
                                            (SW_w + s_ + 1) * 128],
                                rhs=asel[:], start=True, stop=True)
                        adst_sb = em.tile([128, SW_w * 2], f32, tag="adst")
                        nc.scalar.activation(adst_sb[:], pe_all[:], AF.Copy)
                        lg = em.tile([128, SW_w, 2], f32, tag="lg")
                        nc.vector.tensor_tensor(
                            out=lg[:], in0=g[:, :, 258:260],
                            in1=adst_sb[:].rearrange("p (s h) -> p s h", h=2),
                            op=OP.add)
                        lr = em.tile([128, SW_w, 2], f32, tag="lr")
                        nc.vector.scalar_tensor_tensor(
                            out=lr[:], in0=lg[:], scalar=NEG_SLOPE,
                            in1=lg[:], op0=OP.mult, op1=OP.max)
                        ex = em.tile([128, SW_w * 2], f32, tag="ex")
                        nc.scalar.activation(
                            ex[:].rearrange("p (s h) -> p s h", h=2),
                            lr[:], AF.Exp)

                        gs = eg.tile([128, SW_w, 258], bf16, tag="gs")
                        pagg = aggp.tile([128, 258], f32, tag="agg")
                        for s_ in range(SW_w):
                            if s_ % 2 == 0:
                                nc.scalar.activation(
                                    gs[:, s_, 0:129], g[:, s_, 0:129],
                                    AF.Copy, scale=ex[:, 2 * s_:2 * s_ + 1])
                                nc.vector.tensor_scalar(
                                    out=gs[:, s_, 129:258],
                                    in0=g[:, s_, 129:258],
                                    scalar1=ex[:, 2 * s_ + 1:2 * s_ + 2],
                                    scalar2=None, op0=OP.mult)
                            else:
                                nc.vector.tensor_scalar(
                                    out=gs[:, s_, 0:129],
                                    in0=g[:, s_, 0:129],
                                    scalar1=ex[:, 2 * s_:2 * s_ + 1],
                                    scalar2=None, op0=OP.mult)
                                nc.scalar.activation(
                                    gs[:, s_, 129:258], g[:, s_, 129:258],
                                    AF.Copy,
                                    scale=ex[:, 2 * s_ + 1:2 * s_ + 2])
                            nc.tensor.matmul(
                                pagg[:], lhsT=ind_sb[:, s_ * 128:
                                                     (s_ + 1) * 128],
                                rhs=gs[:, s_, :], start=(s_ == 0),
                                stop=(s_ == SW_w - 1))

                        # combine: h_new = 0.5*(msg0/den0 + msg1/den1) + bias
                        rs = em.tile([128, 2], f32, tag="rs")
                        nc.scalar.activation(
                            rs[:], pagg[:].rearrange(
                                "p (a b) -> p a b", b=129)[:, :, 128:129],
                            AF.Copy, bias=1e-16)
                        nc.vector.reciprocal(rs[:], rs[:])
                        nc.vector.tensor_scalar(out=rs[:], in0=rs[:],
                                                scalar1=0.5, scalar2=None,
                                                op0=OP.mult)
                        t0 = em.tile([128, 128], f32, tag="t0")
                        nc.scalar.activation(t0[:], pagg[:, 0:128], AF.Copy,
                                             scale=rs[:, 0:1])
                        t1 = em.tile([128, 128], f32, tag="t1")
                        nc.scalar.activation(t1[:], pagg[:, 129:257],
                                             AF.Copy, scale=rs[:, 1:2])
                        h01 = em.tile([128, 128], f32, tag="h01")
                        nc.vector.tensor_tensor(out=h01[:], in0=t0[:],
                                                in1=t1[:], op=OP.add)
                        if conv < NUM_CONVS - 1:
                            hn = em.tile([128, 128], f32, tag="hn")
                            nc.vector.tensor_tensor(out=hn[:], in0=h01[:],
                                                    in1=bias_sb[:], op=OP.add)
                            pt = etp.tile([128, 128], f32, tag="pt")
                            nc.tensor.transpose(pt[:], hn[:], ident[:])
                            ht = em.tile([128, 128], bf16, tag="ht")
                            nc.vector.tensor_copy(ht[:], pt[:])
                            nc.sync.dma_start(
                                out=h_shT[:, 128 * w:128 * w + nwn],
                                in_=ht[:, 0:nwn])
                        else:
                            hn = em.tile([128, 128], bf16, tag="hnf")
                            nc.vector.tensor_tensor(out=hn[:], in0=h01[:],
                                                    in1=bias_sb[:], op=OP.add)
                            nc.sync.dma_start(
                                out=h_sh[128 * w:128 * w + nwn, :],
                                in_=hn[0:nwn, :])

                if conv < NUM_CONVS - 1:
                    nc.gpsimd.collective_compute(
                        "AllGather", mybir.AluOpType.bypass,
                        ins=[h_shT[:]], outs=[ag_hT[:]],
                        replica_groups=[list(range(NCORES))])
                else:
                    nc.gpsimd.collective_compute(
                        "AllGather", mybir.AluOpType.bypass,
                        ins=[h_sh[:]], outs=[h3_full[:]],
                        replica_groups=[list(range(NCORES))])

            # ---- set2set on this core's 16-graph slice ----
            with tc.tile_pool(name="s2s", bufs=1) as sp, \
                 tc.tile_pool(name="s2w", bufs=2) as swp, \
                 tc.tile_pool(name="s2p", bufs=2, space="PSUM") as s2p, \
                 tc.tile_pool(name="s2g", bufs=1, space="PSUM") as s2g:
                xloc = sp.tile([128, T, 128], f32)
                gxa = sp.tile([128, T, 128], bf16)
                gxb = sp.tile([128, T, 128], bf16)
                xia = sp.tile([128, T * 8], i16)
                nc.sync.dma_start(out=xia[:], in_=s2s_xidxA[:])
                xib = sp.tile([128, T * 8], i16)
                nc.sync.dma_start(out=xib[:], in_=s2s_xidxB[:])
                srep = sp.tile([128, T * 128], bf16)
                nc.sync.dma_start(out=srep[:], in_=s2s_selrep[:])
                for (gx, xi, r0, r1) in ((gxa, xia, 0, HALF),
                                         (gxb, xib, HALF, N)):
                    s0 = 0
                    while s0 < T:
                        ns = min(8, T - s0)
                        nc.gpsimd.dma_gather(
                            out_ap=gx[:, s0:s0 + ns, :],
                            in_ap=h3_full[r0:r1, :],
                            idxs_ap=xi[:, s0 * 8:(s0 + ns) * 8],
                            num_idxs=ns * 128,
                            num_idxs_reg=ns * 128, elem_size=128,
                            elem_step=128, queue_num=qc())
                        s0 += ns
                srv = srep[:].rearrange("p (t d) -> p t d", d=128)
                nc.vector.tensor_tensor(out=xloc[:], in0=gxa[:], in1=gxb[:],
                                        op=OP.subtract)
                nc.vector.tensor_tensor(out=xloc[:], in0=xloc[:], in1=srv,
                                        op=OP.mult)
                nc.vector.tensor_tensor(out=xloc[:], in0=xloc[:], in1=gxb[:],
                                        op=OP.add)
                bl = sp.tile([128, T], f32)
                nc.sync.dma_start(out=bl[:],
                                  in_=s2s_bloc.rearrange("t p o -> p (t o)"))
                brep_sb = sp.tile([16, T, 128], f32)
                nc.sync.dma_start(out=brep_sb[:],
                                  in_=s2s_brep.rearrange("t p d -> p t d"))
                oh = sp.tile([128, T, 16], f32)
                ohT = sp.tile([16, T, 128], f32)
                for t in range(T):
                    nc.vector.tensor_scalar(
                        out=oh[:, t, :], in0=iota16_row[:],
                        scalar1=bl[:, t:t + 1], scalar2=None, op0=OP.is_equal)
                    nc.vector.tensor_scalar(
                        out=ohT[:, t, :], in0=brep_sb[:, t, :],
                        scalar1=iota16_col[:], scalar2=None, op0=OP.is_equal)

                qT = sp.tile([128, 16], f32)
                nc.vector.memset(qT[:], 0.0)
                rT = sp.tile([128, 16], f32)
                nc.vector.memset(rT[:], 0.0)
                cst = sp.tile([16, 128], f32)
                nc.vector.memset(cst[:], 0.0)
                eloc = sp.tile([128, T], f32)

                for step in range(AGGR_STEPS):
                    pg = s2g.tile([16, 512], f32, tag="acc")
                    nc.tensor.matmul(pg[:], lhsT=qT[:], rhs=wia[:],
                                     start=True, stop=False)
                    nc.tensor.matmul(pg[:], lhsT=rT[:], rhs=wib[:],
                                     start=False, stop=False)
                    nc.tensor.matmul(pg[:], lhsT=qT[:], rhs=whh[:],
                                     start=False, stop=True)
                    pg_sb = swp.tile([16, 512], f32, tag="pgsb")
                    nc.scalar.activation(pg_sb[:], pg[:], AF.Copy)
                    gt = swp.tile([16, 512], f32, tag="gt")
                    nc.vector.tensor_tensor(out=gt[:], in0=pg_sb[:],
                                            in1=bg_sb[:], op=OP.add)
                    sf = swp.tile([16, 128], f32, tag="sf")
                    nc.scalar.activation(sf[:], gt[:, 128:256], AF.Sigmoid)
                    si_ = swp.tile([16, 128], f32, tag="si")
                    nc.scalar.activation(si_[:], gt[:, 0:128], AF.Sigmoid)
                    tg = swp.tile([16, 128], f32, tag="tg")
                    nc.scalar.activation(tg[:], gt[:, 256:384], AF.Tanh)
                    so = swp.tile([16, 128], f32, tag="so")
                    nc.scalar.activation(so[:], gt[:, 384:512], AF.Sigmoid)
                    c2 = swp.tile([16, 128], f32, tag="c2")
                    nc.vector.tensor_tensor(out=c2[:], in0=sf[:], in1=cst[:],
                                            op=OP.mult)
                    it_ = swp.tile([16, 128], f32, tag="it")
                    nc.vector.tensor_tensor(out=it_[:], in0=si_[:], in1=tg[:],
                                            op=OP.mult)
                    nc.vector.tensor_tensor(out=c2[:], in0=c2[:], in1=it_[:],
                                            op=OP.add)
                    nc.vector.tensor_copy(cst[:], c2[:])
                    tc2 = swp.tile([16, 128], f32, tag="tc2")
                    nc.scalar.activation(tc2[:], c2[:], AF.Tanh)
                    qpad = swp.tile([128, 128], f32, tag="qpad")
                    nc.vector.memset(qpad[:], 0.0)
                    nc.vector.tensor_tensor(out=qpad[0:16, :], in0=so[:],
                                            in1=tc2[:], op=OP.mult)
                    ptq = s2p.tile([128, 128], f32, tag="tp")
                    nc.tensor.transpose(ptq[:], qpad[:], ident[:])
                    nc.vector.tensor_copy(qT[:], ptq[:, 0:16])

                    # e_n = x_n . q[batch_n]
                    for t in range(T):
                        pqx = s2p.tile([128, 128], f32, tag="tp")
                        nc.tensor.matmul(pqx[:], lhsT=ohT[:, t, :],
                                         rhs=qpad[0:16, :], start=True,
                                         stop=True)
                        pqs = swp.tile([128, 128], f32, tag="pqs")
                        nc.scalar.activation(pqs[:], pqx[:], AF.Copy)
                        xq = swp.tile([128, 128], f32, tag="xq")
                        nc.vector.tensor_tensor(out=xq[:], in0=xloc[:, t, :],
                                                in1=pqs[:], op=OP.mult)
                        nc.vector.tensor_reduce(
                            out=eloc[:, t:t + 1], in_=xq[:],
                            axis=mybir.AxisListType.X, op=OP.add)
                    # global (per-core) max for stability
                    mx = swp.tile([128, 1], f32, tag="mx")
                    nc.vector.tensor_reduce(out=mx[:], in_=eloc[:],
                                            axis=mybir.AxisListType.X,
                                            op=OP.max)
                    mpad = swp.tile([128, 128], f32, tag="mpad")
                    nc.vector.memset(mpad[:], -1e30)
                    nc.vector.tensor_copy(mpad[:, 0:1], mx[:])
                    ptm = s2p.tile([128, 128], f32, tag="tp")
                    nc.tensor.transpose(ptm[:], mpad[:], ident[:])
                    msc = swp.tile([1, 1], f32, tag="msc")
                    nc.vector.tensor_reduce(out=msc[:], in_=ptm[0:1, :],
                                            axis=mybir.AxisListType.X,
                                            op=OP.max)
                    pnm = s2p.tile([128, 1], f32, tag="tp")
                    nc.tensor.matmul(pnm[:], lhsT=negones_row[:], rhs=msc[:],
                                     start=True, stop=True)
                    negm = swp.tile([128, 1], f32, tag="negm")
                    nc.vector.tensor_copy(negm[:], pnm[:])

                    pr = s2g.tile([16, 129], f32, tag="acc")
                    for t in range(T):
                        ev = swp.tile([128, 1], f32, tag="ev")
                        nc.scalar.activation(ev[:], eloc[:, t:t + 1], AF.Exp,
                                             bias=negm[:, 0:1])
                        msg = swp.tile([128, 129], f32, tag="msg")
                        nc.scalar.activation(msg[:, 0:128], xloc[:, t, :],
                                             AF.Copy, scale=ev[:, 0:1])
                        nc.vector.tensor_copy(msg[:, 128:129], ev[:])
                        nc.tensor.matmul(pr[:], lhsT=oh[:, t, :], rhs=msg[:],
                                         start=(t == 0), stop=(t == T - 1))
                    rsum = swp.tile([16, 1], f32, tag="rsum")
                    nc.scalar.activation(rsum[:], pr[:, 128:129], AF.Copy,
                                         bias=1e-16)
                    nc.vector.reciprocal(rsum[:], rsum[:])
                    rpad = swp.tile([128, 128], f32, tag="rpad")
                    nc.vector.memset(rpad[:], 0.0)
                    nc.scalar.activation(rpad[0:16, :], pr[:, 0:128],
                                         AF.Copy, scale=rsum[:, 0:1])
                    ptr = s2p.tile([128, 128], f32, tag="tp")
                    nc.tensor.transpose(ptr[:], rpad[:], ident[:])
                    nc.vector.tensor_copy(rT[:], ptr[:, 0:16])

                # MLP head
                pm1 = s2g.tile([16, 128], f32, tag="acc")
                nc.tensor.matmul(pm1[:], lhsT=qT[:], rhs=w1a_sb[:],
                                 start=True, stop=False)
                nc.tensor.matmul(pm1[:], lhsT=rT[:], rhs=w1b_sb[:],
                                 start=False, stop=True)
                hidp = swp.tile([128, 128], f32, tag="hidp")
                nc.vector.memset(hidp[:], 0.0)
                nc.vector.tensor_tensor(out=hidp[0:16, :], in0=pm1[:],
                                        in1=b1_sb[:], op=OP.add)
                nc.scalar.activation(hidp[0:16, :], hidp[0:16, :], AF.Relu)
                pth = s2p.tile([128, 128], f32, tag="tp")
                nc.tensor.transpose(pth[:], hidp[:], ident[:])
                hT_m = swp.tile([128, 16], f32, tag="hTm")
                nc.vector.tensor_copy(hT_m[:], pth[:, 0:16])
                pm2 = s2g.tile([16, 128], f32, tag="acc")
                nc.tensor.matmul(pm2[:], lhsT=hT_m[:], rhs=w2_sb[:],
                                 start=True, stop=True)
                osb = swp.tile([16, 128], f32, tag="osb")
                nc.vector.tensor_tensor(out=osb[:], in0=pm2[:], in1=b2_sb[:],
                                        op=OP.add)
                nc.sync.dma_start(out=out[:], in_=osb[:])

    nc.compile()
    _fix_swdge_queues(nc)
    if not int(os.environ.get("K_NOSPLIT", "0")):
        _split_waits(nc)
    return nc


def _fix_swdge_queues(nc):
    """queue_num must match the DMASW lane assigned (in final scheduled
    order) by tile_sem_assignment: lane L -> queue L % num_queues."""
    from concourse.tile_sem_assignment import PROC_NAME_TO_IDX
    from concourse import mybir
    lane_of = {PROC_NAME_TO_IDX[f"DMASW{i}"]: i for i in range(8)}
    n = 0
    for f in nc.m.functions:
        for bb in f.blocks:
            for ins in bb.instructions:
                proc = getattr(ins, "bass_scheduled_proc", None)
                if proc in lane_of and hasattr(ins, "queue_num"):
                    qn = lane_of[proc] % nc.num_swdge_queues
                    if ins.queue_num != qn:
                        ins.queue_num = qn
                        n += 1
    return n


# ---------------------------------------------------------------- entry
def kernel(x, edge_index, edge_attr, batch_index,
           gat_W, gat_att_src, gat_att_dst, gat_bias,
           lstm_Wih, lstm_Whh, lstm_bih, lstm_bhh,
           mlp_W1, mlp_b1, mlp_W2, mlp_b2, _trace=False):
    del edge_attr
    x = np.asarray(x, np.float32)
    edge_index = np.asarray(edge_index)
    batch_index = np.asarray(batch_index)

    cfg, per_core = _host_prep(x, edge_index, batch_index,
                               gat_W, gat_att_src, gat_att_dst)

    Wih = np.asarray(lstm_Wih, np.float32)     # [512, 256]
    Whh = np.asarray(lstm_Whh, np.float32)     # [512, 128]
    WihT = Wih.T.copy()                        # [256, 512]
    bias_gates = (np.asarray(lstm_bih, np.float32)
                  + np.asarray(lstm_bhh, np.float32))
    common = dict(
        h0T=cfg["h0T"], W_eff=cfg["W_eff"],
        bias_rep=np.tile(np.asarray(gat_bias, np.float32)[None, :],
                         (128, 1)),
        WihT_a=WihT[0:128], WihT_b=WihT[128:256],
        WhhT=Whh.T.copy(),
        bg_rep=np.tile(bias_gates[None, :], (16, 1)),
        W1a=np.asarray(mlp_W1, np.float32)[0:128],
        W1b=np.asarray(mlp_W1, np.float32)[128:256],
        W2=np.asarray(mlp_W2, np.float32),
        b1_rep=np.tile(np.asarray(mlp_b1, np.float32)[None, :], (16, 1)),
        b2_rep=np.tile(np.asarray(mlp_b2, np.float32)[None, :], (16, 1)),
    )

    key = (tuple(cfg["SA"]), tuple(cfg["SB"]), cfg["T"])
    if _cached.get("key") != key:
        _cached["nc"] = _build(cfg)
        _cached["key"] = key
    nc = _cached["nc"]

    in_maps = []
    for c in range(NCORES):
        m = dict(common)
        m.update(per_core[c])
        m = {k: np.ascontiguousarray(v) for k, v in m.items()}
        in_maps.append(m)

    from concourse.bass_utils import run_bass_kernel_spmd
    res = run_bass_kernel_spmd(nc, in_maps, core_ids=list(range(NCORES)),
                               trace=_trace)
    outp = np.concatenate([res.results[c]["out"] for c in range(NCORES)],
                          axis=0)
    if _trace:
        _cached["last_exec_ns"] = res.exec_time_ns
        _cached["last_res"] = res
    return outp
